# revision 13
# baseline (speedup 1.0000x reference)
"""Banded-attention (AttentionWindow) TRN2 kernel, data-parallel over batch on 8 NeuronCores.

Reference computation (per batch b):
  Q = x @ W;  scores = Q @ x^T;  scores[|i-j| > 64] = -1e9
  probs = softmax(scores, axis=-1);  out = x + relu(probs @ x)

Kernel strategy:
  - One batch per core (batch=8, n_cores=8), W replicated. No collectives.
  - All matmuls in bf16 (fp32 PSUM accumulation). Inputs are cast to bf16
    (round-to-nearest-even) on the host during sharding; the output is also
    bf16 (upcast to f32 on the host), halving write traffic.
  - The |i-j| <= 64 band means each 128-query tile attends to an exact
    256-wide, 64-aligned key window. A second, 64-row-shifted DMA of x (xs)
    makes the PV contraction exactly 2x128 key chunks for every tile.
  - Softmax is fp32 (DVE mask+max, ACT exp with per-partition bias and sum
    accumulation, bf16 probs out); normalization is folded into the final
    per-partition scale of the ReLU.
  - probs are transposed on the PE (bf16 identity matmul) for PV; the
    transpose PSUM shares the scores bank ring via bitcast.
  - DMA layout (HWDGE rings serialize one transfer at a time ~110GB/s;
    the gpsimd SWDGE ring pipelines descriptors at full HBM rate): the
    entire input stream goes on SWDGE in strict need order, so per-ring
    FIFO replaces semaphore gates.  W is issued as column slabs (cols
    0:256 for all d first, then 256:1024) so the first Q-projection
    e-chunks unblock several us before the full 2MB of W has landed.
    x^T group 0 rides the two otherwise-idle HWDGE rings in parallel.
    Outputs go on sync (tile 15 split sync/scalar for latency).
  - The HAM power controller grants full PE clock only after ~3.4us of
    sustained activity: zero-matmuls are interleaved between the DMA-paced
    first e-chunk's accumulation steps so the PE stays busy from the
    post-barrier point until the input stream saturates it.
  - Per-tile elementwise work is split across vector+scalar (gpsimd only
    issues DMAs, plus the last tiles' second residual-add half) so the
    final tiles' drain chain is short.

Inputs: repr [8, 2048, 1024] f32, W [1024, 1024] f32.
Output: [8, 2048, 1024] f32.
"""
from contextlib import ExitStack

import numpy as np

SEQ, HID = 2048, 1024
W2 = 64                  # window half-width
QTL = 128                # queries per softmax tile
KX = 256                 # exact key window per q-tile
NQ = SEQ // QTL          # 16
GQ = 512                 # queries per Q^T-projection group
NG = SEQ // GQ           # 4
ND = HID // 128          # 8 contraction chunks
NEG = -1e9
LEAD = 3                 # head->tail pipeline depth


def _legalize_waits(nc):
    """This walrus build accepts 1 sync wait per instruction (2 on
    EventSemaphore). Hoist excess waits onto EventSemaphore prefixes on the
    same engine."""
    from concourse import mybir

    n = 0
    for func in nc.m.functions:
        for blk in func.blocks:
            out = []
            changed = False
            for inst in list(blk.instructions):
                si = inst.sync_info
                cap = 2 if isinstance(inst, mybir.InstEventSemaphore) else 1
                if si is not None and len(si.on_wait) > cap:
                    waits = list(si.on_wait)
                    for i in range(cap, len(waits), 2):
                        ev = mybir.InstEventSemaphore(
                            name=f"{inst.name}_waitfix{i}",
                            engine=inst.engine,
                            ins=[],
                            outs=[],
                            sync_info=mybir.SyncInfo(on_wait=waits[i:i + 2],
                                                     on_update=[]),
                        )
                        out.append(ev)
                        n += 1
                    inst.sync_info = mybir.SyncInfo(on_wait=waits[:cap],
                                                    on_update=list(si.on_update))
                    changed = True
                out.append(inst)
            if changed:
                blk.instructions = out
    return n


def _build(nc):
    import concourse.tile as tile
    from concourse import masks, mybir

    F32 = mybir.dt.float32
    BF16 = mybir.dt.bfloat16
    AF = mybir.ActivationFunctionType
    ALU = mybir.AluOpType
    X = mybir.AxisListType.X

    w = nc.dram_tensor("w", [HID, HID], BF16, kind="ExternalInput").ap()
    xt = nc.dram_tensor("xt", [HID, SEQ], BF16, kind="ExternalInput").ap()
    xn = nc.dram_tensor("xn", [SEQ, HID], BF16, kind="ExternalInput").ap()
    out = nc.dram_tensor("out", [SEQ, HID], BF16, kind="ExternalOutput").ap()

    with tile.TileContext(nc) as tc, ExitStack() as ctx:
        pool = ctx.enter_context(tc.tile_pool(name="sb", bufs=1))
        ps = ctx.enter_context(tc.tile_pool(name="ps", bufs=1, space="PSUM"))

        # ---- persistent bf16 operands ----
        wtb = [pool.tile([128, HID], BF16, tag=f"w{d}", name=f"w{d}")
               for d in range(ND)]
        xttb = [pool.tile([128, SEQ], BF16, tag=f"xt{d}", name=f"xt{d}")
                for d in range(ND)]
        xnb = [pool.tile([128, HID], BF16, tag=f"xn{k}", name=f"xn{k}")
               for k in range(NQ)]
        xsb = [pool.tile([128, HID], BF16, tag=f"xs{j}", name=f"xs{j}")
               for j in range(NQ - 1)]

        # ---- warm-up tiles: memset on DVE so no DMA/gpsimd dependency ----
        zer128 = pool.tile([128, 128], BF16, tag="z128", name="z128")
        zer512 = pool.tile([128, 512], BF16, tag="z512", name="z512")
        nc.vector.memset(zer128[:], 0.0)
        nc.vector.memset(zer512[:], 0.0)

        # ---- x^T group 0 on the two HWDGE rings (parallel to SWDGE W) ----
        for d in range(ND):
            eng = nc.sync if d % 2 == 0 else nc.scalar
            eng.dma_start(xttb[d][:, 0:GQ], xt[128 * d:128 * (d + 1), 0:GQ])

        # ---- SWDGE input stream, strict need order ----
        # W column slabs: cols 0:256 of every row chunk first, so e-chunks
        # 0-1 unblock as soon as ~0.5MB has landed.
        for d in range(ND):
            nc.gpsimd.dma_start(wtb[d][:, 0:256], w[128 * d:128 * (d + 1), 0:256])
        for d in range(ND):
            nc.gpsimd.dma_start(wtb[d][:, 256:HID],
                                w[128 * d:128 * (d + 1), 256:HID])

        def stage_xt_group(g):
            for d in range(ND):
                nc.gpsimd.dma_start(xttb[d][:, GQ * g:GQ * (g + 1)],
                                    xt[128 * d:128 * (d + 1), GQ * g:GQ * (g + 1)])

        def dma_xn(k):
            nc.gpsimd.dma_start(xnb[k][:], xn[128 * k:128 * (k + 1), :])

        def dma_xs(k):
            nc.gpsimd.dma_start(xsb[k][:], xn[64 + 128 * k:64 + 128 * (k + 1), :])

        stage_xt_group(1)

        # warm PSUM tile borrows the (idle until the first tail) ra ring so
        # fillers never touch the Q-projection accumulation banks.
        warm = ps.tile([128, 512], F32, tag="ra", bufs=3, name="warm")

        def filler(n):
            # 256-col zero matmuls: fine-grained so a filler overrunning a
            # data-ready boundary wastes at most ~0.2us of PE time
            for _ in range(n):
                nc.tensor.matmul(warm[:, 0:256], zer128[:], zer512[:, 0:256],
                                 start=True, stop=True)

        # one unbroken ~3.4us block right after the barrier secures the HAM
        # full-clock grant by ~11us; afterwards the sub-1.5us DMA-paced idles
        # of the first e-chunk cannot revoke it, so no more fillers needed.
        filler(16)

        # ---- identity + banded masks (keep iff |r + off - c| <= W2) ----
        # gpsimd ops; emitted after the critical W/g1 issues so they do not
        # delay the SWDGE stream (masks are not needed until the first head).
        idn = pool.tile([128, 128], F32, tag="idn", name="idn")
        masks.make_identity(nc, idn[:])
        idnb = pool.tile([128, 128], BF16, tag="idnb", name="idnb")
        nc.vector.tensor_copy(idnb[:], idn[:])
        zer512f = pool.tile([128, 512], F32, tag="z512f", name="z512f")
        nc.vector.memset(zer512f[:], 0.0)
        mask_by_off = {}
        for off in (0, 64, 128):
            m = pool.tile([128, KX], F32, tag=f"mask{off}", name=f"mask{off}")
            nc.gpsimd.memset(m[:], 0.0)
            nc.gpsimd.affine_select(out=m[:], in_=m[:], compare_op=ALU.is_ge,
                                    fill=NEG, base=W2 - off, channel_multiplier=-1,
                                    pattern=[[1, KX]])
            nc.gpsimd.affine_select(out=m[:], in_=m[:], compare_op=ALU.is_ge,
                                    fill=NEG, base=W2 + off, channel_multiplier=1,
                                    pattern=[[-1, KX]])
            mask_by_off[off] = m

        # rest of the SWDGE stream in need order
        dma_xn(0)
        dma_xn(1)
        dma_xs(0)
        stage_xt_group(2)
        dma_xs(1)
        dma_xn(2)
        dma_xs(2)
        stage_xt_group(3)
        dma_xn(3)
        for k in range(3, NQ - 1):
            dma_xs(k)
            dma_xn(k + 1)

        qt_sb = {}

        def emit_qt_e(g, e, tiles):
            pq = ps.tile([128, GQ], F32, tag=f"q{e % 2}", bufs=1,
                         name=f"qtp{g}_{e}")
            for d in range(ND):
                nc.tensor.matmul(pq[:], wtb[d][:, 128 * e:128 * (e + 1)],
                                 xttb[d][:, GQ * g:GQ * (g + 1)],
                                 start=(d == 0), stop=(d == ND - 1))
            st = pool.tile([128, GQ], BF16, tag=f"qt{e}", bufs=2,
                           name=f"qt{g}_{e}")
            # split the PSUM->SBUF copy across both engines: halves run in
            # parallel, halving the latency before dependent scores matmuls
            nc.vector.tensor_copy(st[:, 0:GQ // 2], pq[:, 0:GQ // 2])
            nc.scalar.copy(st[:, GQ // 2:GQ], pq[:, GQ // 2:GQ])
            tiles.append(st)

        def emit_qt_half(g, half):
            tiles = qt_sb.setdefault(g, [])
            for e in range(4 * half, 4 * half + 4):
                emit_qt_e(g, e, tiles)

        state = {}

        def emit_head(i):
            g = i // (GQ // QTL)
            qloc = (i % (GQ // QTL)) * QTL
            kx = min(max(128 * i - W2, 0), SEQ - KX)   # exact scores window
            off = 128 * i - kx
            sp = ps.tile([128, KX], F32, tag="s", bufs=3, name=f"s{i}")
            for e in range(ND):
                nc.tensor.matmul(sp[:], qt_sb[g][e][:, qloc:qloc + QTL],
                                 xttb[e][:, kx:kx + KX],
                                 start=(e == 0), stop=(e == ND - 1))
            sm = pool.tile([128, KX], F32, tag="sm", bufs=4, name=f"sm{i}")
            nc.vector.tensor_tensor(out=sm[:], in0=sp[:], in1=mask_by_off[off][:],
                                    op=ALU.add)
            negmax = pool.tile([128, 1], F32, tag="nm", bufs=4, name=f"nm{i}")
            nc.vector.tensor_reduce(negmax[:], sm[:], axis=X, op=ALU.max,
                                    negate=True)
            probs = pool.tile([128, KX], BF16, tag="pb", bufs=4, name=f"pb{i}")
            sums = pool.tile([128, 1], F32, tag="sums", bufs=4, name=f"sums{i}")
            nc.scalar.activation(probs[:], sm[:], AF.Exp, bias=negmax[:],
                                 scale=1.0, accum_out=sums[:])
            recip = pool.tile([128, 1], F32, tag="recip", bufs=4, name=f"recip{i}")
            nc.vector.reciprocal(recip[:], sums[:])
            state[i] = (probs, recip)

        def emit_tail(i):
            probs, recip = state.pop(i)
            # transpose PSUM shares the scores bank ring (bitcast to bf16)
            tps = ps.tile([128, KX], F32, tag="s", bufs=3, name=f"tps{i}")
            tp = tps[:].bitcast(BF16)
            for j in range(KX // 128):
                nc.tensor.transpose(tp[:, 128 * j:128 * (j + 1)],
                                    probs[:, 128 * j:128 * (j + 1)], idnb[:])
            probsT = pool.tile([128, KX], BF16, tag="pt", bufs=3, name=f"pt{i}")
            nc.vector.tensor_copy(probsT[:, 0:128], tp[:, 0:128])
            nc.scalar.copy(probsT[:, 128:KX], tp[:, 128:KX])
            if i == 0:
                rhs = (xnb[0], xnb[1])
            elif i == NQ - 1:
                rhs = (xnb[NQ - 2], xnb[NQ - 1])
            else:
                rhs = (xsb[i - 1], xsb[i])
            ot = pool.tile([128, HID], BF16, tag="ot", bufs=3, name=f"ot{i}")
            rr = pool.tile([128, HID], BF16, tag="rr", bufs=3, name=f"rr{i}")
            cw = HID // 2
            if i == NQ - 1:
                # last tile: quarter-pipelined PV -> relu(ACT) -> add(DVE)
                # so the post-PE drain chain is ~1us shorter
                qw = HID // 4
                for h in range(2):
                    ra = ps.tile([128, GQ], F32, tag="ra", bufs=3,
                                 name=f"ra{i}_{h}")
                    for q in range(2):
                        cq = slice(qw * (2 * h + q), qw * (2 * h + q + 1))
                        rq = slice(qw * q, qw * (q + 1))
                        for j in range(2):
                            nc.tensor.matmul(ra[:, rq],
                                             probsT[:, 128 * j:128 * (j + 1)],
                                             rhs[j][:, cq],
                                             start=(j == 0), stop=(j == 1))
                        nc.scalar.activation(rr[:, cq], ra[:, rq], AF.Relu,
                                             bias=0.0, scale=recip[:])
                        nc.vector.tensor_tensor(out=ot[:, cq], in0=rr[:, cq],
                                                in1=xnb[i][:, cq], op=ALU.add)
                    oeng = nc.sync if h == 0 else nc.scalar
                    oeng.dma_start(out[128 * i:128 * (i + 1),
                                       cw * h:cw * (h + 1)],
                                   ot[:, cw * h:cw * (h + 1)])
                return
            rap = []
            for h in range(2):
                ra = ps.tile([128, GQ], F32, tag="ra", bufs=3, name=f"ra{i}_{h}")
                for j in range(2):
                    nc.tensor.matmul(ra[:, 0:cw],
                                     probsT[:, 128 * j:128 * (j + 1)],
                                     rhs[j][:, cw * h:cw * (h + 1)],
                                     start=(j == 0), stop=(j == 1))
                rap.append(ra)
            c0, c1 = slice(0, cw), slice(cw, HID)
            # relu halves in parallel on ACT/DVE; both adds on DVE (bf16).
            # gpsimd does NO elementwise work at all, so it reaches its slow
            # (~8us) end-of-kernel SWDGE drain right after its DMA issues
            # (~40us) and the drain hides completely mid-stream.
            nc.scalar.activation(rr[:, c0], rap[0][:, 0:cw], AF.Relu,
                                 bias=0.0, scale=recip[:])
            nc.vector.scalar_tensor_tensor(out=rr[:, c1], in0=rap[1][:, 0:cw],
                                           scalar=recip[:],
                                           in1=zer512f[:],
                                           op0=ALU.mult, op1=ALU.max)
            nc.vector.tensor_tensor(out=ot[:, c0], in0=rr[:, c0],
                                    in1=xnb[i][:, c0], op=ALU.add)
            nc.vector.tensor_tensor(out=ot[:, c1], in0=rr[:, c1],
                                    in1=xnb[i][:, c1], op=ALU.add)
            nc.sync.dma_start(out[128 * i:128 * (i + 1), :], ot[:])

        # emit group g's Q halves 2-3 tiles before its first head needs it
        HSCHED = {2: (1, 0), 3: (1, 1), 6: (2, 0), 7: (2, 1),
                  10: (3, 0), 11: (3, 1)}

        def emit_for_head(j):
            if j in HSCHED:
                emit_qt_half(*HSCHED[j])
            emit_head(j)

        emit_qt_half(0, 0)
        emit_qt_half(0, 1)
        for j in range(LEAD):
            emit_for_head(j)
        for i in range(LEAD, NQ + LEAD):
            if i < NQ:
                emit_for_head(i)
            emit_tail(i - LEAD)

    return nc


def _to_bf16(x):
    """Round-to-nearest-even f32 -> bf16, returned as an ml_dtypes.bfloat16
    array (what jax/bass expect for bf16 dram tensors)."""
    import ml_dtypes

    u = np.ascontiguousarray(x, dtype=np.float32).view(np.uint32)
    r = ((u + 0x7FFF + ((u >> 16) & 1)) >> 16).astype(np.uint16)
    return r.view(ml_dtypes.bfloat16)


def _run(x_all, W, trace=False, tmpdir=None, trace_cores=None):
    import concourse.bass as bass
    from concourse import bass_utils

    nc = bass.Bass("TRN2", target_bir_lowering=False, debug=False, num_devices=8)
    _build(nc)
    _legalize_waits(nc)

    Wb = _to_bf16(W)
    in_maps = []
    for c in range(8):
        in_maps.append({
            "w": Wb,
            "xt": _to_bf16(np.ascontiguousarray(x_all[c].T)),
            "xn": _to_bf16(x_all[c]),
        })
    kwargs = {}
    if trace:
        kwargs = dict(trace=True, tmpdir=tmpdir,
                      trace_cores=trace_cores if trace_cores is not None else [0])
    res = bass_utils.run_bass_kernel_spmd(nc, in_maps, core_ids=list(range(8)),
                                          **kwargs)
    out = np.stack([np.asarray(r["out"]).astype(np.float32)
                    for r in res.results])
    return out, res


def kernel(repr, W):
    x_all = np.ascontiguousarray(np.asarray(repr, dtype=np.float32))
    Wm = np.ascontiguousarray(np.asarray(W, dtype=np.float32))
    out, _ = _run(x_all, Wm, trace=False)
    return out


# Alias for external drivers that expect a `build(nc)` entry point.
build = _build
